# revision 6
# baseline (speedup 1.0000x reference)
"""Trainium2 Bass kernel for nn_DeepPSDual (masked-decay GRU + gather head).

Contract: kernel(**inputs) takes FULL unsharded inputs (as produced by the
problem's setup_inputs) and returns the full outputs
(eta, H_raw, H_agg, mask_sel) exactly like the reference.

Sharding: pure data parallel over batch B=256 -> 8 cores x 32 batches.
All weights replicated; the T=512 recurrence stays local per core.

Device layout trick: everything on device lives in [feature-on-partitions,
(t*32+b) on free] layout; the host does all transposes (cheap numpy) so the
device never transposes anything.

Key algebra used (exact):
  delta = exp(-softplus(dt)) == sigmoid(-dt)
  1 - delta == sigmoid(dt)
  M_raw is a 0/1 mask  =>  x_dec == x_hat  (both equal m*x + (1-m)*x_mean)
  h_pre_t = delta_t*h_{t-1} + c_t,   c_t = (1-delta_t)*h_til_t
  gh_t = h_pre_t @ Whh.T + bhh  -> fed as h_pre @ WhhT accumulated onto
  PSUM-resident precomputed gate inputs (gi parts + biases).
"""

import os
import sys
from contextlib import ExitStack

import numpy as np

for _p in ("/opt/trn_rl_repo", "/root/.axon_site/_ro/trn_rl_repo"):
    if os.path.isdir(_p) and _p not in sys.path:
        sys.path.insert(0, _p)

import concourse.bass as bass  # noqa: E402
import concourse.bacc as bacc  # noqa: E402
import concourse.tile as tile  # noqa: E402
from concourse import mybir  # noqa: E402
from concourse.bass_utils import run_bass_kernel_spmd  # noqa: E402
from concourse import library_config  # noqa: E402

F32 = mybir.dt.float32
AF = mybir.ActivationFunctionType

NCORES = 8
B = 256
BLOC = B // NCORES  # 32
T = 512
P = 128  # feature dim == hidden dim
H = 128
TA = 64  # aggregated slots
P_STD = 32
P_STATIC = 16
HEAD_H = 64
W = 16  # time steps per window
NCOL = T * BLOC  # 16384 columns per core
PAD = 32  # zero columns appended to H for masked gather
NW = T // W

_BUILD_CACHE = {}


def _build(x_mean_zero: bool):
    nc = bacc.Bacc("TRN2", debug=False)

    # ---- DRAM I/O (per core) ----
    d_x = nc.dram_tensor("x", [P, NCOL], F32, kind="ExternalInput")
    d_m = nc.dram_tensor("m", [P, NCOL], F32, kind="ExternalInput")
    d_dt = nc.dram_tensor("dt", [P, NCOL], F32, kind="ExternalInput")
    d_wdT = nc.dram_tensor("wdT", [P, H], F32, kind="ExternalInput")
    d_wih = {}
    for g in ("r", "z", "n"):
        for c in ("x", "m"):
            d_wih[g, c] = nc.dram_tensor(f"wih_{g}{c}", [P, H], F32, kind="ExternalInput")
    d_whhT = {g: nc.dram_tensor(f"whhT_{g}", [H, H], F32, kind="ExternalInput") for g in ("r", "z", "n")}
    d_bias_r = nc.dram_tensor("bias_r", [1, H], F32, kind="ExternalInput")  # bih_r+bhh_r
    d_bias_z = nc.dram_tensor("bias_z", [1, H], F32, kind="ExternalInput")
    d_bhh_n = nc.dram_tensor("bhh_n", [1, H], F32, kind="ExternalInput")
    d_bih_n = nc.dram_tensor("bih_n", [H, 1], F32, kind="ExternalInput")
    d_bd = nc.dram_tensor("bd", [H, 1], F32, kind="ExternalInput")
    d_xmean = nc.dram_tensor("xmean", [P, 1], F32, kind="ExternalInput")
    d_gidx = nc.dram_tensor("gidx", [P, (BLOC * TA) // 16], mybir.dt.int16, kind="ExternalInput")
    d_w1hT = nc.dram_tensor("w1hT", [H, HEAD_H], F32, kind="ExternalInput")
    d_w1sT = nc.dram_tensor("w1sT", [P_STD, HEAD_H], F32, kind="ExternalInput")
    d_w1zT = nc.dram_tensor("w1zT", [P_STATIC, HEAD_H], F32, kind="ExternalInput")
    d_b1 = nc.dram_tensor("b1", [1, HEAD_H], F32, kind="ExternalInput")
    d_w2T = nc.dram_tensor("w2T", [HEAD_H, 1], F32, kind="ExternalInput")
    d_stdT = nc.dram_tensor("stdT", [P_STD, BLOC * TA], F32, kind="ExternalInput")
    d_zT = nc.dram_tensor("zT", [P_STATIC, BLOC * TA], F32, kind="ExternalInput")

    d_hout = nc.dram_tensor("h_out", [H, NCOL], F32, kind="ExternalOutput")
    d_hagg = nc.dram_tensor("hagg_out", [H, BLOC * TA], F32, kind="ExternalOutput")
    d_eta = nc.dram_tensor("eta_out", [1, BLOC * TA], F32, kind="ExternalOutput")

    with tile.TileContext(nc) as tc, ExitStack() as ctx:
        singles = ctx.enter_context(tc.tile_pool(name="singles", bufs=1))

        # persistent SBUF tensors
        H_sb = singles.tile([H, NCOL + PAD], F32)
        nc.vector.memset(H_sb[:, NCOL:], 0.0)

        s_wdT = singles.tile([P, H], F32)
        nc.sync.dma_start(s_wdT, d_wdT[:])
        s_wih = {}
        for k, d in d_wih.items():
            s_wih[k] = singles.tile([P, H], F32, name=f"wih_{k[0]}{k[1]}", tag=f"wih_{k[0]}{k[1]}")
            nc.sync.dma_start(s_wih[k], d[:])
        s_whhT = {}
        for g, d in d_whhT.items():
            s_whhT[g] = singles.tile([H, H], F32, name=f"whhT_{g}", tag=f"whhT_{g}")
            nc.sync.dma_start(s_whhT[g], d[:])
        s_bias_r = singles.tile([1, H], F32)
        nc.sync.dma_start(s_bias_r, d_bias_r[:])
        s_bias_z = singles.tile([1, H], F32)
        nc.sync.dma_start(s_bias_z, d_bias_z[:])
        s_bhh_n = singles.tile([1, H], F32)
        nc.sync.dma_start(s_bhh_n, d_bhh_n[:])
        s_bih_n = singles.tile([H, 1], F32)
        nc.sync.dma_start(s_bih_n, d_bih_n[:])
        s_bd = singles.tile([H, 1], F32)
        nc.sync.dma_start(s_bd, d_bd[:])
        s_xmean = singles.tile([P, 1], F32)
        nc.sync.dma_start(s_xmean, d_xmean[:])
        s_ones = singles.tile([1, W * BLOC], F32)
        nc.vector.memset(s_ones, 1.0)

        WIN = W * BLOC  # 512 columns per window

        with ExitStack() as chain_ctx:
            inp_pool = chain_ctx.enter_context(tc.tile_pool(name="inp", bufs=2))
            mid_pool = chain_ctx.enter_context(tc.tile_pool(name="mid", bufs=2))
            ps_rz = chain_ctx.enter_context(tc.tile_pool(name="ps_rz", bufs=2, space="PSUM"))
            ps_n = chain_ctx.enter_context(tc.tile_pool(name="ps_n", bufs=2, space="PSUM"))
            ps_tmp = chain_ctx.enter_context(tc.tile_pool(name="ps_tmp", bufs=2, space="PSUM"))
            st_pool = chain_ctx.enter_context(tc.tile_pool(name="step", bufs=3))

            def precompute(w):
                """DMA + bulk precompute for window w; returns tiles the chain needs."""
                c0 = w * WIN
                x_w = inp_pool.tile([P, WIN], F32, tag="x")
                m_w = inp_pool.tile([P, WIN], F32, tag="m")
                dt_w = inp_pool.tile([P, WIN], F32, tag="dt")
                nc.sync.dma_start(x_w, d_x[:, c0:c0 + WIN])
                nc.sync.dma_start(m_w, d_m[:, c0:c0 + WIN])
                nc.sync.dma_start(dt_w, d_dt[:, c0:c0 + WIN])

                delta = mid_pool.tile([P, WIN], F32, tag="delta")
                sdt = mid_pool.tile([P, WIN], F32, tag="sdt")
                nc.scalar.activation(delta, dt_w, AF.Sigmoid, scale=-1.0)
                nc.scalar.activation(sdt, dt_w, AF.Sigmoid)

                xhat = mid_pool.tile([P, WIN], F32, tag="xhat")
                if x_mean_zero:
                    nc.vector.tensor_mul(xhat, m_w, x_w)
                else:
                    t1 = mid_pool.tile([P, WIN], F32, tag="xc")
                    nc.vector.tensor_scalar(t1, x_w, s_xmean[:, :], None, mybir.AluOpType.subtract)
                    nc.vector.tensor_mul(t1, m_w, t1)
                    nc.vector.tensor_scalar(xhat, t1, s_xmean[:, :], None, mybir.AluOpType.add)

                # h_til = tanh(Wd @ xhat + bd)
                htp = ps_tmp.tile([H, WIN], F32, tag="pst")
                nc.tensor.matmul(htp, s_wdT, xhat, start=True, stop=True, skip_group_check=True)
                htil = mid_pool.tile([H, WIN], F32, tag="htil")
                nc.scalar.activation(htil, htp, AF.Tanh, bias=s_bd[:, :])
                c_w = mid_pool.tile([H, WIN], F32, tag="c")
                nc.vector.tensor_mul(c_w, sdt, htil)

                # gate-input precompute into PSUM (PE-written so chain can accumulate)
                rz = ps_rz.tile([H, 2, WIN], F32, tag="rz")
                nc.tensor.matmul(rz[:, 0, :], s_bias_r, s_ones, start=True, stop=False, skip_group_check=True)
                nc.tensor.matmul(rz[:, 0, :], s_wih["r", "x"], xhat, start=False, stop=False, skip_group_check=True)
                nc.tensor.matmul(rz[:, 0, :], s_wih["r", "m"], m_w, start=False, stop=False, skip_group_check=True)
                nc.tensor.matmul(rz[:, 1, :], s_bias_z, s_ones, start=True, stop=False, skip_group_check=True)
                nc.tensor.matmul(rz[:, 1, :], s_wih["z", "x"], xhat, start=False, stop=False, skip_group_check=True)
                nc.tensor.matmul(rz[:, 1, :], s_wih["z", "m"], m_w, start=False, stop=False, skip_group_check=True)

                nb = ps_n.tile([H, WIN], F32, tag="nb")
                nc.tensor.matmul(nb, s_bhh_n, s_ones, start=True, stop=False, skip_group_check=True)

                # a_n = gi_n + bih_n (SBUF resident)
                anp = ps_tmp.tile([H, WIN], F32, tag="pst")
                nc.tensor.matmul(anp, s_wih["n", "x"], xhat, start=True, stop=False, skip_group_check=True)
                nc.tensor.matmul(anp, s_wih["n", "m"], m_w, start=False, stop=True, skip_group_check=True)
                a_n = mid_pool.tile([H, WIN], F32, tag="an")
                nc.vector.tensor_scalar(a_n, anp, s_bih_n[:, :], None, mybir.AluOpType.add)

                return delta, c_w, a_n, rz, nb

            def chain(w, tiles):
                delta, c_w, a_n, rz, nb = tiles
                for k in range(W):
                    g = w * W + k  # global step
                    cs, ce = k * BLOC, (k + 1) * BLOC
                    if g == 0:
                        hpre = c_w[:, cs:ce]
                    else:
                        hprev = H_sb[:, (g - 1) * BLOC:g * BLOC]
                        u = st_pool.tile([H, BLOC], F32, tag="u")
                        nc.vector.tensor_mul(u, delta[:, cs:ce], hprev)
                        hpre_t = st_pool.tile([H, BLOC], F32, tag="hpre")
                        nc.vector.tensor_add(hpre_t, u, c_w[:, cs:ce])
                        hpre = hpre_t[:, :]
                    nc.tensor.matmul(rz[:, 0, cs:ce], s_whhT["r"], hpre, start=False, stop=True, skip_group_check=True)
                    nc.tensor.matmul(rz[:, 1, cs:ce], s_whhT["z"], hpre, start=False, stop=True, skip_group_check=True)
                    nc.tensor.matmul(nb[:, cs:ce], s_whhT["n"], hpre, start=False, stop=(k == W - 1), skip_group_check=True)

                    rz_sb = st_pool.tile([H, 2, BLOC], F32, tag="rzsb")
                    nc.scalar.activation(rz_sb, rz[:, :, cs:ce], AF.Sigmoid)
                    r = rz_sb[:, 0, :]
                    z = rz_sb[:, 1, :]

                    tmp = st_pool.tile([H, BLOC], F32, tag="tmp")
                    nc.vector.tensor_mul(tmp, r, nb[:, cs:ce])
                    wn = st_pool.tile([H, BLOC], F32, tag="wn")
                    nc.vector.tensor_add(wn, tmp, a_n[:, cs:ce])
                    n_sb = st_pool.tile([H, BLOC], F32, tag="nsb")
                    nc.scalar.activation(n_sb, wn, AF.Tanh)

                    dd = st_pool.tile([H, BLOC], F32, tag="dd")
                    nc.vector.tensor_sub(dd, hpre, n_sb)
                    ee = st_pool.tile([H, BLOC], F32, tag="ee")
                    nc.vector.tensor_mul(ee, z, dd)
                    nc.vector.tensor_add(H_sb[:, g * BLOC:(g + 1) * BLOC], n_sb, ee)

                # stream this window of H out
                nc.sync.dma_start(d_hout[:, w * WIN:(w + 1) * WIN], H_sb[:, w * WIN:(w + 1) * WIN])

            tiles = precompute(0)
            for w in range(NW):
                nxt = precompute(w + 1) if w + 1 < NW else None
                chain(w, tiles)
                tiles = nxt

        # ---- H_agg gather (gpsimd) ----
        NAGG = BLOC * TA  # 2048
        hagg_sb = singles.tile([H, NAGG], F32)
        s_gidx = singles.tile([P, NAGG // 16], mybir.dt.int16)
        nc.sync.dma_start(s_gidx, d_gidx[:])
        nc.gpsimd.load_library(library_config.ap_gather)
        nc.gpsimd.ap_gather(hagg_sb[:, :], H_sb[:, :], s_gidx[:, :],
                            channels=P, num_elems=NCOL + PAD, d=1, num_idxs=NAGG)
        nc.sync.dma_start(d_hagg[:], hagg_sb)

        # ---- head: eta = relu(feats @ W1.T + b1) @ W2.T ----
        s_w1hT = singles.tile([H, HEAD_H], F32)
        nc.sync.dma_start(s_w1hT, d_w1hT[:])
        s_w1sT = singles.tile([P_STD, HEAD_H], F32)
        nc.sync.dma_start(s_w1sT, d_w1sT[:])
        s_w1zT = singles.tile([P_STATIC, HEAD_H], F32)
        nc.sync.dma_start(s_w1zT, d_w1zT[:])
        s_b1 = singles.tile([1, HEAD_H], F32)
        nc.sync.dma_start(s_b1, d_b1[:])
        s_w2T = singles.tile([HEAD_H, 1], F32)
        nc.sync.dma_start(s_w2T, d_w2T[:])
        s_stdT = singles.tile([P_STD, NAGG], F32)
        nc.sync.dma_start(s_stdT, d_stdT[:])
        s_zT = singles.tile([P_STATIC, NAGG], F32)
        nc.sync.dma_start(s_zT, d_zT[:])
        s_ones_h = singles.tile([1, 512], F32)
        nc.vector.memset(s_ones_h, 1.0)

        r1_sb = singles.tile([HEAD_H, NAGG], F32)
        eta_sb = singles.tile([1, NAGG], F32)
        with ExitStack() as hctx:
            ps_head = hctx.enter_context(tc.tile_pool(name="ps_head", bufs=2, space="PSUM"))
            ps_eta = hctx.enter_context(tc.tile_pool(name="ps_eta", bufs=2, space="PSUM"))
            for j in range(NAGG // 512):
                sl = slice(j * 512, (j + 1) * 512)
                hp = ps_head.tile([HEAD_H, 512], F32, tag="hp")
                nc.tensor.matmul(hp, s_b1, s_ones_h, start=True, stop=False, skip_group_check=True)
                nc.tensor.matmul(hp, s_w1hT, hagg_sb[:, sl], start=False, stop=False, skip_group_check=True)
                nc.tensor.matmul(hp, s_w1sT, s_stdT[:, sl], start=False, stop=False, skip_group_check=True)
                nc.tensor.matmul(hp, s_w1zT, s_zT[:, sl], start=False, stop=True, skip_group_check=True)
                nc.scalar.activation(r1_sb[:, sl], hp, AF.Relu)
                ep = ps_eta.tile([1, 512], F32, tag="ep")
                nc.tensor.matmul(ep, s_w2T, r1_sb[:, sl], start=True, stop=True, skip_group_check=True)
                nc.vector.tensor_copy(eta_sb[:, sl], ep)
        nc.sync.dma_start(d_eta[:], eta_sb)

    nc.finalize()
    return nc


def _get_nc(x_mean_zero: bool):
    key = ("v1", x_mean_zero)
    if key not in _BUILD_CACHE:
        _BUILD_CACHE[key] = _build(x_mean_zero)
    return _BUILD_CACHE[key]


def _prep(X_raw, M_raw, DT_raw, STD_agg, Z, idx_map, x_mean,
          Wd, bd, Wih, bih, Whh, bhh, W1, b1, W2):
    X_raw = np.ascontiguousarray(np.asarray(X_raw, dtype=np.float32))
    M_raw = np.ascontiguousarray(np.asarray(M_raw, dtype=np.float32))
    DT_raw = np.ascontiguousarray(np.asarray(DT_raw, dtype=np.float32))
    STD_agg = np.asarray(STD_agg, dtype=np.float32)
    Z = np.asarray(Z, dtype=np.float32)
    idx_np = np.asarray(idx_map)
    x_mean = np.asarray(x_mean, dtype=np.float32)
    Wd = np.asarray(Wd, dtype=np.float32)
    bd = np.asarray(bd, dtype=np.float32)
    Wih = np.asarray(Wih, dtype=np.float32)
    bih = np.asarray(bih, dtype=np.float32)
    Whh = np.asarray(Whh, dtype=np.float32)
    bhh = np.asarray(bhh, dtype=np.float32)
    W1 = np.asarray(W1, dtype=np.float32)
    b1 = np.asarray(b1, dtype=np.float32)
    W2 = np.asarray(W2, dtype=np.float32)

    x_mean_zero = not np.any(x_mean)

    bih_r, bih_z, bih_n = bih[:H], bih[H:2 * H], bih[2 * H:]
    bhh_r, bhh_z, bhh_n = bhh[:H], bhh[H:2 * H], bhh[2 * H:]
    shared = {
        "wdT": np.ascontiguousarray(Wd.T),
        "wih_rx": np.ascontiguousarray(Wih[0:H, :P].T),
        "wih_rm": np.ascontiguousarray(Wih[0:H, P:].T),
        "wih_zx": np.ascontiguousarray(Wih[H:2 * H, :P].T),
        "wih_zm": np.ascontiguousarray(Wih[H:2 * H, P:].T),
        "wih_nx": np.ascontiguousarray(Wih[2 * H:, :P].T),
        "wih_nm": np.ascontiguousarray(Wih[2 * H:, P:].T),
        "whhT_r": np.ascontiguousarray(Whh[0:H].T),
        "whhT_z": np.ascontiguousarray(Whh[H:2 * H].T),
        "whhT_n": np.ascontiguousarray(Whh[2 * H:].T),
        "bias_r": np.ascontiguousarray((bih_r + bhh_r)[None, :]),
        "bias_z": np.ascontiguousarray((bih_z + bhh_z)[None, :]),
        "bhh_n": np.ascontiguousarray(bhh_n[None, :]),
        "bih_n": np.ascontiguousarray(bih_n[:, None]),
        "bd": np.ascontiguousarray(bd[:, None]),
        "xmean": np.ascontiguousarray(x_mean[:, None]),
        "w1hT": np.ascontiguousarray(W1[:, :H].T),
        "w1sT": np.ascontiguousarray(W1[:, H:H + P_STD].T),
        "w1zT": np.ascontiguousarray(W1[:, H + P_STD:].T),
        "b1": np.ascontiguousarray(b1[None, :]),
        "w2T": np.ascontiguousarray(W2.T),
    }

    in_maps = []
    for c in range(NCORES):
        bs = slice(c * BLOC, (c + 1) * BLOC)
        xc = np.ascontiguousarray(X_raw[bs].transpose(2, 1, 0).reshape(P, NCOL))
        mc = np.ascontiguousarray(M_raw[bs].transpose(2, 1, 0).reshape(P, NCOL))
        dtc = np.ascontiguousarray(DT_raw[bs].transpose(2, 1, 0).reshape(P, NCOL))
        idxc = idx_np[bs].astype(np.int64)
        valid = idxc >= 0
        safe = np.clip(idxc, 0, T - 1)
        cols = safe * BLOC + np.arange(BLOC)[:, None]
        cols = np.where(valid, cols, NCOL).astype(np.int64).reshape(-1)
        gidx16 = np.zeros((16, len(cols) // 16), dtype=np.int16)
        for j, v in enumerate(cols):
            gidx16[j % 16, j // 16] = v
        gidx = np.ascontiguousarray(np.tile(gidx16, (8, 1)))
        stdT = np.ascontiguousarray(STD_agg[bs].transpose(2, 0, 1).reshape(P_STD, BLOC * TA))
        zT = np.ascontiguousarray(
            np.repeat(Z[bs].T[:, :, None], TA, axis=2).reshape(P_STATIC, BLOC * TA))
        im = {"x": xc, "m": mc, "dt": dtc, "gidx": gidx, "stdT": stdT, "zT": zT}
        im.update(shared)
        in_maps.append(im)
    return in_maps, idx_np, x_mean_zero


def _post(outs, idx_np):
    H_raw = np.empty((B, T, H), dtype=np.float32)
    H_agg = np.empty((B, TA, H), dtype=np.float32)
    eta = np.empty((B, TA), dtype=np.float32)
    for c in range(NCORES):
        bs = slice(c * BLOC, (c + 1) * BLOC)
        ho = outs[c]["h_out"].reshape(H, T, BLOC)
        H_raw[bs] = ho.transpose(2, 1, 0)
        ha = outs[c]["hagg_out"].reshape(H, BLOC, TA)
        H_agg[bs] = ha.transpose(1, 2, 0)
        eta[bs] = outs[c]["eta_out"].reshape(BLOC, TA)
    mask_sel = (idx_np >= 0).astype(np.float32)
    return eta, H_raw, H_agg, mask_sel


def kernel(**inputs):
    in_maps, idx_np, x_mean_zero = _prep(**inputs)
    nc = _get_nc(x_mean_zero)
    res = run_bass_kernel_spmd(nc, in_maps, list(range(NCORES)))
    return _post(res.results, idx_np)


# revision 9
# speedup vs baseline: 51.0874x; 51.0874x over previous
"""Trainium2 Bass kernel for nn_DeepPSDual (masked-decay GRU + gather head).

Contract: kernel(**inputs) takes FULL unsharded inputs (as produced by the
problem's setup_inputs) and returns the full outputs
(eta, H_raw, H_agg, mask_sel) exactly like the reference.

Sharding: pure data parallel over batch B=256 -> 8 cores x 32 batches.
All weights replicated; the T=512 recurrence stays local per core.

Device layout trick: everything on device lives in [feature-on-partitions,
(t*32+b) on free] layout; the host does all transposes (cheap numpy) so the
device never transposes anything.

Key algebra used (exact):
  delta = exp(-softplus(dt)) == sigmoid(-dt)
  1 - delta == sigmoid(dt)
  M_raw is a 0/1 mask  =>  x_dec == x_hat  (both equal m*x + (1-m)*x_mean)
  h_pre_t = delta_t*h_{t-1} + c_t,   c_t = (1-delta_t)*h_til_t
  gh_t = h_pre_t @ Whh.T + bhh  -> fed as h_pre @ WhhT accumulated onto
  PSUM-resident precomputed gate inputs (gi parts + biases).
"""

import os
import sys
from contextlib import ExitStack

import numpy as np

for _p in ("/opt/trn_rl_repo", "/root/.axon_site/_ro/trn_rl_repo"):
    if os.path.isdir(_p) and _p not in sys.path:
        sys.path.insert(0, _p)

import concourse.bass as bass  # noqa: E402
import concourse.bacc as bacc  # noqa: E402
import concourse.tile as tile  # noqa: E402
from concourse import mybir  # noqa: E402
from concourse.bass_utils import run_bass_kernel_spmd  # noqa: E402
from concourse import library_config  # noqa: E402

F32 = mybir.dt.float32
AF = mybir.ActivationFunctionType

NCORES = 8
B = 256
BLOC = B // NCORES  # 32
T = 512
P = 128  # feature dim == hidden dim
H = 128
TA = 64  # aggregated slots
P_STD = 32
P_STATIC = 16
HEAD_H = 64
W = 16  # bulk precompute window (steps)
WP = 8  # PSUM window (steps per bank set)
NCOL = T * BLOC  # 16384 columns per core
PAD = 32  # zero columns appended to H for masked gather
NW = T // W

_BUILD_CACHE = {}


def _build(x_mean_zero: bool):
    nc = bacc.Bacc("TRN2", debug=False)

    # ---- DRAM I/O (per core) ----
    d_x = nc.dram_tensor("x", [P, NCOL], F32, kind="ExternalInput")
    d_m = nc.dram_tensor("m", [P, NCOL], F32, kind="ExternalInput")
    d_dt = nc.dram_tensor("dt", [P, NCOL], F32, kind="ExternalInput")
    d_wdT = nc.dram_tensor("wdT", [P, H], F32, kind="ExternalInput")
    d_wih = {}
    for g in ("r", "z", "n"):
        for c in ("x", "m"):
            d_wih[g, c] = nc.dram_tensor(f"wih_{g}{c}", [P, H], F32, kind="ExternalInput")
    d_whhT = {g: nc.dram_tensor(f"whhT_{g}", [H, H], F32, kind="ExternalInput") for g in ("r", "z", "n")}
    d_bias3 = nc.dram_tensor("bias3", [3, H], F32, kind="ExternalInput")  # [bih_r+bhh_r; bih_z+bhh_z; bhh_n]
    d_ind3 = nc.dram_tensor("ind3", [3, 3 * WP * 16], F32, kind="ExternalInput")  # section indicators
    d_bih_n = nc.dram_tensor("bih_n", [H, 1], F32, kind="ExternalInput")
    d_bd = nc.dram_tensor("bd", [H, 1], F32, kind="ExternalInput")
    d_xmean = nc.dram_tensor("xmean", [P, 1], F32, kind="ExternalInput")
    d_gidx = nc.dram_tensor("gidx", [P, (BLOC * TA) // 16], mybir.dt.int16, kind="ExternalInput")
    d_w1hT = nc.dram_tensor("w1hT", [H, HEAD_H], F32, kind="ExternalInput")
    d_w1sT = nc.dram_tensor("w1sT", [P_STD, HEAD_H], F32, kind="ExternalInput")
    d_w1zT = nc.dram_tensor("w1zT", [P_STATIC, HEAD_H], F32, kind="ExternalInput")
    d_b1 = nc.dram_tensor("b1", [1, HEAD_H], F32, kind="ExternalInput")
    d_w2T = nc.dram_tensor("w2T", [HEAD_H, 1], F32, kind="ExternalInput")
    d_stdT = nc.dram_tensor("stdT", [P_STD, BLOC * TA], F32, kind="ExternalInput")
    d_zT = nc.dram_tensor("zT", [P_STATIC, BLOC * TA], F32, kind="ExternalInput")

    d_hout = nc.dram_tensor("h_out", [H, NCOL], F32, kind="ExternalOutput")
    d_hagg = nc.dram_tensor("hagg_out", [H, BLOC * TA], F32, kind="ExternalOutput")
    d_eta = nc.dram_tensor("eta_out", [1, BLOC * TA], F32, kind="ExternalOutput")

    with tile.TileContext(nc) as tc, ExitStack() as ctx:
        singles = ctx.enter_context(tc.tile_pool(name="singles", bufs=1))

        # persistent SBUF tensors
        H_sb = singles.tile([H, NCOL + PAD], F32)
        nc.vector.memset(H_sb[:, NCOL:], 0.0)

        s_wdT = singles.tile([P, H], F32)
        nc.sync.dma_start(s_wdT, d_wdT[:])
        s_wih = {}
        for k, d in d_wih.items():
            s_wih[k] = singles.tile([P, H], F32, name=f"wih_{k[0]}{k[1]}", tag=f"wih_{k[0]}{k[1]}")
            nc.sync.dma_start(s_wih[k], d[:])
        s_whhT = {}
        for g, d in d_whhT.items():
            s_whhT[g] = singles.tile([H, H], F32, name=f"whhT_{g}", tag=f"whhT_{g}")
            nc.sync.dma_start(s_whhT[g], d[:])
        s_bias3 = singles.tile([3, H], F32)
        nc.sync.dma_start(s_bias3, d_bias3[:])
        s_ind3 = singles.tile([3, 3 * WP * 16], F32)
        nc.sync.dma_start(s_ind3, d_ind3[:])
        s_bih_n = singles.tile([H, 1], F32)
        nc.sync.dma_start(s_bih_n, d_bih_n[:])
        s_bd = singles.tile([H, 1], F32)
        nc.sync.dma_start(s_bd, d_bd[:])
        s_xmean = singles.tile([P, 1], F32)
        nc.sync.dma_start(s_xmean, d_xmean[:])

        WIN = W * BLOC  # 512 columns per bulk window
        HB = BLOC // 2  # 16 batches per half-chain
        NSEC = WP * HB  # 128 columns per gate section in a PSUM bank

        with ExitStack() as chain_ctx:
            inp_pool = chain_ctx.enter_context(tc.tile_pool(name="inp", bufs=2))
            mid_pool = chain_ctx.enter_context(tc.tile_pool(name="mid", bufs=2))
            ps_cbA = chain_ctx.enter_context(tc.tile_pool(name="ps_cbA", bufs=2, space="PSUM"))
            ps_cbB = chain_ctx.enter_context(tc.tile_pool(name="ps_cbB", bufs=2, space="PSUM"))
            ps_tmp = chain_ctx.enter_context(tc.tile_pool(name="ps_tmp", bufs=2, space="PSUM"))
            st_pool = chain_ctx.enter_context(tc.tile_pool(name="step", bufs=3))

            def precompute(w):
                """DMA + bulk elementwise for bulk window w (W steps)."""
                c0 = w * WIN
                x_w = inp_pool.tile([P, WIN], F32, tag="x")
                m_w = inp_pool.tile([P, WIN], F32, tag="m")
                dt_w = inp_pool.tile([P, WIN], F32, tag="dt")
                nc.sync.dma_start(x_w, d_x[:, c0:c0 + WIN])
                nc.sync.dma_start(m_w, d_m[:, c0:c0 + WIN])
                nc.sync.dma_start(dt_w, d_dt[:, c0:c0 + WIN])

                delta = mid_pool.tile([P, WIN], F32, tag="delta")
                sdt = mid_pool.tile([P, WIN], F32, tag="sdt")
                nc.scalar.activation(delta, dt_w, AF.Sigmoid, scale=-1.0)
                nc.scalar.activation(sdt, dt_w, AF.Sigmoid)

                xhat = mid_pool.tile([P, WIN], F32, tag="xhat")
                if x_mean_zero:
                    nc.vector.tensor_mul(xhat, m_w, x_w)
                else:
                    t1 = mid_pool.tile([P, WIN], F32, tag="xc")
                    nc.vector.tensor_scalar(t1, x_w, s_xmean[:, :], None, mybir.AluOpType.subtract)
                    nc.vector.tensor_mul(t1, m_w, t1)
                    nc.vector.tensor_scalar(xhat, t1, s_xmean[:, :], None, mybir.AluOpType.add)

                # h_til = tanh(Wd @ xhat + bd); c = (1-delta)*h_til
                htp = ps_tmp.tile([H, WIN // 2], F32, tag="pst")
                htp2 = ps_tmp.tile([H, WIN // 2], F32, tag="pst")
                nc.tensor.matmul(htp, s_wdT, xhat[:, :WIN // 2], start=True, stop=True, skip_group_check=True)
                nc.tensor.matmul(htp2, s_wdT, xhat[:, WIN // 2:], start=True, stop=True, skip_group_check=True)
                htil = mid_pool.tile([H, WIN], F32, tag="htil")
                nc.scalar.activation(htil[:, :WIN // 2], htp, AF.Tanh, bias=s_bd[:, :])
                nc.scalar.activation(htil[:, WIN // 2:], htp2, AF.Tanh, bias=s_bd[:, :])
                c_w = mid_pool.tile([H, WIN], F32, tag="c")
                nc.vector.tensor_mul(c_w, sdt, htil)

                # a_n = gi_n + bih_n (SBUF resident, full window)
                anp = ps_tmp.tile([H, WIN // 2], F32, tag="pst")
                anp2 = ps_tmp.tile([H, WIN // 2], F32, tag="pst")
                nc.tensor.matmul(anp, s_wih["n", "x"], xhat[:, :WIN // 2], start=True, stop=False, skip_group_check=True)
                nc.tensor.matmul(anp, s_wih["n", "m"], m_w[:, :WIN // 2], start=False, stop=True, skip_group_check=True)
                nc.tensor.matmul(anp2, s_wih["n", "x"], xhat[:, WIN // 2:], start=True, stop=False, skip_group_check=True)
                nc.tensor.matmul(anp2, s_wih["n", "m"], m_w[:, WIN // 2:], start=False, stop=True, skip_group_check=True)
                a_n = mid_pool.tile([H, WIN], F32, tag="an")
                nc.vector.tensor_scalar(a_n[:, :WIN // 2], anp, s_bih_n[:, :], None, mybir.AluOpType.add)
                nc.vector.tensor_scalar(a_n[:, WIN // 2:], anp2, s_bih_n[:, :], None, mybir.AluOpType.add)
                return delta, c_w, a_n, xhat, m_w

            def psum_windows(tiles, j):
                """Gate-input matmuls for PSUM window j (WP steps) of a bulk window."""
                delta, c_w, a_n, xhat, m_w = tiles
                # [P, W, 2, HB] views of the bulk tiles
                xv = xhat.rearrange("p (t h b) -> p t h b", t=W, h=2)
                mv = m_w.rearrange("p (t h b) -> p t h b", t=W, h=2)
                cv = c_w.rearrange("p (t h b) -> p t h b", t=W, h=2)
                t0 = j * WP
                cbs = []
                for h, pool in ((0, ps_cbA), (1, ps_cbB)):
                    cb = pool.tile([H, 3, NSEC], F32, tag=f"cb{h}", name=f"cb{h}")
                    xs = xv[:, t0:t0 + WP, h, :]
                    ms = mv[:, t0:t0 + WP, h, :]
                    cs_ = cv[:, t0:t0 + WP, h, :]
                    nc.tensor.matmul(cb[:, :, :], s_bias3, s_ind3, start=True, stop=False, skip_group_check=True)
                    nc.tensor.matmul(cb[:, 0, :], s_wih["r", "x"], xs, start=False, stop=False, skip_group_check=True)
                    nc.tensor.matmul(cb[:, 0, :], s_wih["r", "m"], ms, start=False, stop=False, skip_group_check=True)
                    nc.tensor.matmul(cb[:, 0, :], s_whhT["r"], cs_, start=False, stop=False, skip_group_check=True)
                    nc.tensor.matmul(cb[:, 1, :], s_wih["z", "x"], xs, start=False, stop=False, skip_group_check=True)
                    nc.tensor.matmul(cb[:, 1, :], s_wih["z", "m"], ms, start=False, stop=False, skip_group_check=True)
                    nc.tensor.matmul(cb[:, 1, :], s_whhT["z"], cs_, start=False, stop=False, skip_group_check=True)
                    nc.tensor.matmul(cb[:, 2, :], s_whhT["n"], cs_, start=False, stop=False, skip_group_check=True)
                    cbs.append(cb)
                return cbs

            def chain(w, j, tiles, cbs):
                """WP serial steps for PSUM window (w, j), two half-chains."""
                delta, c_w, a_n, xhat, m_w = tiles
                for k in range(WP):
                    g = (w * W) + j * WP + k  # global step
                    col = (j * WP + k) * BLOC  # column offset in bulk tiles
                    us = []
                    hpres = []
                    for h in (0, 1):
                        cs = col + h * HB
                        if g == 0:
                            us.append(None)
                            hpres.append(c_w[:, cs:cs + HB])
                        else:
                            hprev = H_sb[:, (g - 1) * BLOC + h * HB:(g - 1) * BLOC + h * HB + HB]
                            u = st_pool.tile([H, HB], F32, tag=f"u{h}", name=f"u{h}")
                            nc.vector.tensor_mul(u, delta[:, cs:cs + HB], hprev)
                            us.append(u)
                    # matmuls grouped per stationary (A then B)
                    if g != 0:
                        for gi_, wname in ((0, "r"), (1, "z"), (2, "n")):
                            for h in (0, 1):
                                cb = cbs[h]
                                nc.tensor.matmul(cb[:, gi_, k * HB:(k + 1) * HB], s_whhT[wname], us[h],
                                                 start=False, stop=(k == WP - 1), skip_group_check=True)
                        for h in (0, 1):
                            cs = col + h * HB
                            hpre_t = st_pool.tile([H, HB], F32, tag=f"hp{h}", name=f"hp{h}")
                            nc.vector.tensor_add(hpre_t, us[h], c_w[:, cs:cs + HB])
                            hpres.append(hpre_t[:, :])
                    rz_sbs = []
                    for h in (0, 1):
                        rz_sb = st_pool.tile([H, 2, HB], F32, tag=f"rz{h}", name=f"rz{h}")
                        nc.scalar.activation(rz_sb, cbs[h][:, 0:2, k * HB:(k + 1) * HB], AF.Sigmoid)
                        rz_sbs.append(rz_sb)
                    wns = []
                    for h in (0, 1):
                        cs = col + h * HB
                        tmp = st_pool.tile([H, HB], F32, tag=f"tm{h}", name=f"tm{h}")
                        nc.vector.tensor_mul(tmp, rz_sbs[h][:, 0, :], cbs[h][:, 2, k * HB:(k + 1) * HB])
                        wn = st_pool.tile([H, HB], F32, tag=f"wn{h}", name=f"wn{h}")
                        nc.vector.tensor_add(wn, tmp, a_n[:, cs:cs + HB])
                        wns.append(wn)
                    n_sbs = []
                    for h in (0, 1):
                        n_sb = st_pool.tile([H, HB], F32, tag=f"ns{h}", name=f"ns{h}")
                        nc.scalar.activation(n_sb, wns[h], AF.Tanh)
                        n_sbs.append(n_sb)
                    for h in (0, 1):
                        dd = st_pool.tile([H, HB], F32, tag=f"dd{h}", name=f"dd{h}")
                        nc.vector.tensor_sub(dd, hpres[h], n_sbs[h])
                        ee = st_pool.tile([H, HB], F32, tag=f"ee{h}", name=f"ee{h}")
                        nc.vector.tensor_mul(ee, rz_sbs[h][:, 1, :], dd)
                        nc.vector.tensor_add(H_sb[:, g * BLOC + h * HB:g * BLOC + h * HB + HB], n_sbs[h], ee)

            tiles = precompute(0)
            cbs = psum_windows(tiles, 0)
            for w in range(NW):
                for j in range(W // WP):
                    # emit the NEXT psum-window's precompute before this chain
                    if j + 1 < W // WP:
                        nxt_cbs = psum_windows(tiles, j + 1)
                        nxt_tiles = tiles
                    elif w + 1 < NW:
                        nxt_tiles = precompute(w + 1)
                        nxt_cbs = psum_windows(nxt_tiles, 0)
                    else:
                        nxt_tiles = nxt_cbs = None
                    chain(w, j, tiles, cbs)
                    if j + 1 == W // WP:
                        nc.sync.dma_start(d_hout[:, w * WIN:(w + 1) * WIN], H_sb[:, w * WIN:(w + 1) * WIN])
                    tiles, cbs = nxt_tiles, nxt_cbs

        # ---- H_agg gather (gpsimd) ----
        NAGG = BLOC * TA  # 2048
        hagg_sb = singles.tile([H, NAGG], F32)
        s_gidx = singles.tile([P, NAGG // 16], mybir.dt.int16)
        nc.sync.dma_start(s_gidx, d_gidx[:])
        nc.gpsimd.load_library(library_config.ap_gather)
        nc.gpsimd.ap_gather(hagg_sb[:, :], H_sb[:, :], s_gidx[:, :],
                            channels=P, num_elems=NCOL + PAD, d=1, num_idxs=NAGG)
        nc.sync.dma_start(d_hagg[:], hagg_sb)

        # ---- head: eta = relu(feats @ W1.T + b1) @ W2.T ----
        s_w1hT = singles.tile([H, HEAD_H], F32)
        nc.sync.dma_start(s_w1hT, d_w1hT[:])
        s_w1sT = singles.tile([P_STD, HEAD_H], F32)
        nc.sync.dma_start(s_w1sT, d_w1sT[:])
        s_w1zT = singles.tile([P_STATIC, HEAD_H], F32)
        nc.sync.dma_start(s_w1zT, d_w1zT[:])
        s_b1 = singles.tile([1, HEAD_H], F32)
        nc.sync.dma_start(s_b1, d_b1[:])
        s_w2T = singles.tile([HEAD_H, 1], F32)
        nc.sync.dma_start(s_w2T, d_w2T[:])
        s_stdT = singles.tile([P_STD, NAGG], F32)
        nc.sync.dma_start(s_stdT, d_stdT[:])
        s_zT = singles.tile([P_STATIC, NAGG], F32)
        nc.sync.dma_start(s_zT, d_zT[:])
        s_ones_h = singles.tile([1, 512], F32)
        nc.vector.memset(s_ones_h, 1.0)

        r1_sb = singles.tile([HEAD_H, NAGG], F32)
        eta_sb = singles.tile([1, NAGG], F32)
        with ExitStack() as hctx:
            ps_head = hctx.enter_context(tc.tile_pool(name="ps_head", bufs=2, space="PSUM"))
            ps_eta = hctx.enter_context(tc.tile_pool(name="ps_eta", bufs=2, space="PSUM"))
            for j in range(NAGG // 512):
                sl = slice(j * 512, (j + 1) * 512)
                hp = ps_head.tile([HEAD_H, 512], F32, tag="hp")
                nc.tensor.matmul(hp, s_b1, s_ones_h, start=True, stop=False, skip_group_check=True)
                nc.tensor.matmul(hp, s_w1hT, hagg_sb[:, sl], start=False, stop=False, skip_group_check=True)
                nc.tensor.matmul(hp, s_w1sT, s_stdT[:, sl], start=False, stop=False, skip_group_check=True)
                nc.tensor.matmul(hp, s_w1zT, s_zT[:, sl], start=False, stop=True, skip_group_check=True)
                nc.scalar.activation(r1_sb[:, sl], hp, AF.Relu)
                ep = ps_eta.tile([1, 512], F32, tag="ep")
                nc.tensor.matmul(ep, s_w2T, r1_sb[:, sl], start=True, stop=True, skip_group_check=True)
                nc.vector.tensor_copy(eta_sb[:, sl], ep)
        nc.sync.dma_start(d_eta[:], eta_sb)

    nc.finalize()
    return nc


def _get_nc(x_mean_zero: bool):
    key = ("v1", x_mean_zero)
    if key not in _BUILD_CACHE:
        _BUILD_CACHE[key] = _build(x_mean_zero)
    return _BUILD_CACHE[key]


def _prep(X_raw, M_raw, DT_raw, STD_agg, Z, idx_map, x_mean,
          Wd, bd, Wih, bih, Whh, bhh, W1, b1, W2):
    X_raw = np.ascontiguousarray(np.asarray(X_raw, dtype=np.float32))
    M_raw = np.ascontiguousarray(np.asarray(M_raw, dtype=np.float32))
    DT_raw = np.ascontiguousarray(np.asarray(DT_raw, dtype=np.float32))
    STD_agg = np.asarray(STD_agg, dtype=np.float32)
    Z = np.asarray(Z, dtype=np.float32)
    idx_np = np.asarray(idx_map)
    x_mean = np.asarray(x_mean, dtype=np.float32)
    Wd = np.asarray(Wd, dtype=np.float32)
    bd = np.asarray(bd, dtype=np.float32)
    Wih = np.asarray(Wih, dtype=np.float32)
    bih = np.asarray(bih, dtype=np.float32)
    Whh = np.asarray(Whh, dtype=np.float32)
    bhh = np.asarray(bhh, dtype=np.float32)
    W1 = np.asarray(W1, dtype=np.float32)
    b1 = np.asarray(b1, dtype=np.float32)
    W2 = np.asarray(W2, dtype=np.float32)

    x_mean_zero = not np.any(x_mean)

    bih_r, bih_z, bih_n = bih[:H], bih[H:2 * H], bih[2 * H:]
    bhh_r, bhh_z, bhh_n = bhh[:H], bhh[H:2 * H], bhh[2 * H:]
    shared = {
        "wdT": np.ascontiguousarray(Wd.T),
        "wih_rx": np.ascontiguousarray(Wih[0:H, :P].T),
        "wih_rm": np.ascontiguousarray(Wih[0:H, P:].T),
        "wih_zx": np.ascontiguousarray(Wih[H:2 * H, :P].T),
        "wih_zm": np.ascontiguousarray(Wih[H:2 * H, P:].T),
        "wih_nx": np.ascontiguousarray(Wih[2 * H:, :P].T),
        "wih_nm": np.ascontiguousarray(Wih[2 * H:, P:].T),
        "whhT_r": np.ascontiguousarray(Whh[0:H].T),
        "whhT_z": np.ascontiguousarray(Whh[H:2 * H].T),
        "whhT_n": np.ascontiguousarray(Whh[2 * H:].T),
        "bias3": np.ascontiguousarray(np.stack([bih_r + bhh_r, bih_z + bhh_z, bhh_n])),
        "ind3": np.ascontiguousarray((np.arange(3 * 128)[None, :] // 128 == np.arange(3)[:, None]).astype(np.float32)),
        "bih_n": np.ascontiguousarray(bih_n[:, None]),
        "bd": np.ascontiguousarray(bd[:, None]),
        "xmean": np.ascontiguousarray(x_mean[:, None]),
        "w1hT": np.ascontiguousarray(W1[:, :H].T),
        "w1sT": np.ascontiguousarray(W1[:, H:H + P_STD].T),
        "w1zT": np.ascontiguousarray(W1[:, H + P_STD:].T),
        "b1": np.ascontiguousarray(b1[None, :]),
        "w2T": np.ascontiguousarray(W2.T),
    }

    in_maps = []
    for c in range(NCORES):
        bs = slice(c * BLOC, (c + 1) * BLOC)
        xc = np.ascontiguousarray(X_raw[bs].transpose(2, 1, 0).reshape(P, NCOL))
        mc = np.ascontiguousarray(M_raw[bs].transpose(2, 1, 0).reshape(P, NCOL))
        dtc = np.ascontiguousarray(DT_raw[bs].transpose(2, 1, 0).reshape(P, NCOL))
        idxc = idx_np[bs].astype(np.int64)
        valid = idxc >= 0
        safe = np.clip(idxc, 0, T - 1)
        cols = safe * BLOC + np.arange(BLOC)[:, None]
        cols = np.where(valid, cols, NCOL).astype(np.int64).reshape(-1)
        gidx16 = np.zeros((16, len(cols) // 16), dtype=np.int16)
        for j, v in enumerate(cols):
            gidx16[j % 16, j // 16] = v
        gidx = np.ascontiguousarray(np.tile(gidx16, (8, 1)))
        stdT = np.ascontiguousarray(STD_agg[bs].transpose(2, 0, 1).reshape(P_STD, BLOC * TA))
        zT = np.ascontiguousarray(
            np.repeat(Z[bs].T[:, :, None], TA, axis=2).reshape(P_STATIC, BLOC * TA))
        im = {"x": xc, "m": mc, "dt": dtc, "gidx": gidx, "stdT": stdT, "zT": zT}
        im.update(shared)
        in_maps.append(im)
    return in_maps, idx_np, x_mean_zero


def _post(outs, idx_np):
    H_raw = np.empty((B, T, H), dtype=np.float32)
    H_agg = np.empty((B, TA, H), dtype=np.float32)
    eta = np.empty((B, TA), dtype=np.float32)
    for c in range(NCORES):
        bs = slice(c * BLOC, (c + 1) * BLOC)
        ho = outs[c]["h_out"].reshape(H, T, BLOC)
        H_raw[bs] = ho.transpose(2, 1, 0)
        ha = outs[c]["hagg_out"].reshape(H, BLOC, TA)
        H_agg[bs] = ha.transpose(1, 2, 0)
        eta[bs] = outs[c]["eta_out"].reshape(BLOC, TA)
    mask_sel = (idx_np >= 0).astype(np.float32)
    return eta, H_raw, H_agg, mask_sel


def kernel(**inputs):
    in_maps, idx_np, x_mean_zero = _prep(**inputs)
    nc = _get_nc(x_mean_zero)
    res = run_bass_kernel_spmd(nc, in_maps, list(range(NCORES)))
    return _post(res.results, idx_np)


# revision 16
# speedup vs baseline: 59.3678x; 1.1621x over previous
"""Trainium2 Bass kernel for nn_DeepPSDual (masked-decay GRU + gather head).

Contract: kernel(**inputs) takes FULL unsharded inputs (as produced by the
problem's setup_inputs) and returns the full outputs
(eta, H_raw, H_agg, mask_sel) exactly like the reference.

Sharding: pure data parallel over batch B=256 -> 8 cores x 32 batches.
All weights replicated; the T=512 recurrence stays local per core.

Device layout trick: everything on device lives in [feature-on-partitions,
(t*32+b) on free] layout; the host does all transposes (cheap numpy) so the
device never transposes anything.

Key algebra used (exact):
  delta = exp(-softplus(dt)) == sigmoid(-dt)
  1 - delta == sigmoid(dt)
  M_raw is a 0/1 mask  =>  x_dec == x_hat  (both equal m*x + (1-m)*x_mean)
  h_pre_t = delta_t*h_{t-1} + c_t,   c_t = (1-delta_t)*h_til_t
  gh_t = h_pre_t @ Whh.T + bhh  -> fed as h_pre @ WhhT accumulated onto
  PSUM-resident precomputed gate inputs (gi parts + biases).
"""

import os
import sys
from contextlib import ExitStack

import numpy as np

for _p in ("/opt/trn_rl_repo", "/root/.axon_site/_ro/trn_rl_repo"):
    if os.path.isdir(_p) and _p not in sys.path:
        sys.path.insert(0, _p)

import concourse.bass as bass  # noqa: E402
import concourse.bacc as bacc  # noqa: E402
import concourse.tile as tile  # noqa: E402
from concourse import mybir  # noqa: E402
from concourse.bass_utils import run_bass_kernel_spmd  # noqa: E402
from concourse import library_config  # noqa: E402

F32 = mybir.dt.float32
AF = mybir.ActivationFunctionType

NCORES = 8
B = 256
BLOC = B // NCORES  # 32
T = 512
P = 128  # feature dim == hidden dim
H = 128
TA = 64  # aggregated slots
P_STD = 32
P_STATIC = 16
HEAD_H = 64
W = 16  # bulk precompute window (steps)
WP = 8  # PSUM window (steps per bank set)
NCOL = T * BLOC  # 16384 columns per core
PAD = 32  # zero columns appended to H for masked gather
NW = T // W

_BUILD_CACHE = {}


def _build(x_mean_zero: bool):
    nc = bacc.Bacc("TRN2", debug=False)

    # ---- DRAM I/O (per core) ----
    d_x = nc.dram_tensor("x", [P, NCOL], F32, kind="ExternalInput")
    d_m = nc.dram_tensor("m", [P, NCOL], F32, kind="ExternalInput")
    d_dt = nc.dram_tensor("dt", [P, NCOL], F32, kind="ExternalInput")
    d_wdT = nc.dram_tensor("wdT", [P, H], F32, kind="ExternalInput")
    d_wih = {}
    for g in ("r", "z", "n"):
        for c in ("x", "m"):
            d_wih[g, c] = nc.dram_tensor(f"wih_{g}{c}", [P, H], F32, kind="ExternalInput")
    d_whhT = {g: nc.dram_tensor(f"whhT_{g}", [H, H], F32, kind="ExternalInput") for g in ("r", "z", "n")}
    d_bias3 = nc.dram_tensor("bias3", [3, H], F32, kind="ExternalInput")  # [bih_r+bhh_r; bih_z+bhh_z; bhh_n]
    d_ind3 = nc.dram_tensor("ind3", [3, 3 * WP * 16], F32, kind="ExternalInput")  # section indicators
    d_bih_n = nc.dram_tensor("bih_n", [H, 1], F32, kind="ExternalInput")
    d_bd = nc.dram_tensor("bd", [H, 1], F32, kind="ExternalInput")
    d_xmean = nc.dram_tensor("xmean", [P, 1], F32, kind="ExternalInput")
    d_gidx = nc.dram_tensor("gidx", [P, (BLOC * TA) // 16], mybir.dt.int16, kind="ExternalInput")
    d_w1hT = nc.dram_tensor("w1hT", [H, HEAD_H], F32, kind="ExternalInput")
    d_w1sT = nc.dram_tensor("w1sT", [P_STD, HEAD_H], F32, kind="ExternalInput")
    d_w1zT = nc.dram_tensor("w1zT", [P_STATIC, HEAD_H], F32, kind="ExternalInput")
    d_b1 = nc.dram_tensor("b1", [1, HEAD_H], F32, kind="ExternalInput")
    d_w2T = nc.dram_tensor("w2T", [HEAD_H, 1], F32, kind="ExternalInput")
    d_stdT = nc.dram_tensor("stdT", [P_STD, BLOC * TA], F32, kind="ExternalInput")
    d_zT = nc.dram_tensor("zT", [P_STATIC, BLOC * TA], F32, kind="ExternalInput")

    d_hout = nc.dram_tensor("h_out", [H, NCOL], F32, kind="ExternalOutput")
    d_hagg = nc.dram_tensor("hagg_out", [H, BLOC * TA], F32, kind="ExternalOutput")
    d_eta = nc.dram_tensor("eta_out", [1, BLOC * TA], F32, kind="ExternalOutput")

    with tile.TileContext(nc) as tc, ExitStack() as ctx:
        singles = ctx.enter_context(tc.tile_pool(name="singles", bufs=1))

        # persistent SBUF tensors
        H_sb = singles.tile([H, NCOL + PAD], F32)
        nc.vector.memset(H_sb[:, NCOL:], 0.0)

        s_wdT = singles.tile([P, H], F32)
        nc.sync.dma_start(s_wdT, d_wdT[:])
        s_wih = {}
        for k, d in d_wih.items():
            s_wih[k] = singles.tile([P, H], F32, name=f"wih_{k[0]}{k[1]}", tag=f"wih_{k[0]}{k[1]}")
            nc.sync.dma_start(s_wih[k], d[:])
        s_whhT = {}
        for g, d in d_whhT.items():
            s_whhT[g] = singles.tile([H, H], F32, name=f"whhT_{g}", tag=f"whhT_{g}")
            nc.sync.dma_start(s_whhT[g], d[:])
        s_bias3 = singles.tile([3, H], F32)
        nc.sync.dma_start(s_bias3, d_bias3[:])
        s_ind3 = singles.tile([3, 3 * WP * 16], F32)
        nc.sync.dma_start(s_ind3, d_ind3[:])
        s_bih_n = singles.tile([H, 1], F32)
        nc.sync.dma_start(s_bih_n, d_bih_n[:])
        s_bd = singles.tile([H, 1], F32)
        nc.sync.dma_start(s_bd, d_bd[:])
        s_xmean = singles.tile([P, 1], F32)
        nc.sync.dma_start(s_xmean, d_xmean[:])

        WIN = W * BLOC  # 512 columns per bulk window
        HB = BLOC // 2  # 16 batches per half-chain
        NSEC = WP * HB  # 128 columns per gate section in a PSUM bank

        with ExitStack() as chain_ctx:
            inp_pool = chain_ctx.enter_context(tc.tile_pool(name="inp", bufs=2))
            mid_pool = chain_ctx.enter_context(tc.tile_pool(name="mid", bufs=2))
            ps_cbA = chain_ctx.enter_context(tc.tile_pool(name="ps_cbA", bufs=3, space="PSUM"))
            ps_cbB = chain_ctx.enter_context(tc.tile_pool(name="ps_cbB", bufs=3, space="PSUM"))
            ps_tmp = chain_ctx.enter_context(tc.tile_pool(name="ps_tmp", bufs=2, space="PSUM"))
            st_pool = chain_ctx.enter_context(tc.tile_pool(name="step", bufs=4))

            def precompute(w):
                """DMA + bulk elementwise for bulk window w (W steps)."""
                c0 = w * WIN
                x_w = inp_pool.tile([P, WIN], F32, tag="x")
                m_w = inp_pool.tile([P, WIN], F32, tag="m")
                dt_w = inp_pool.tile([P, WIN], F32, tag="dt")
                nc.sync.dma_start(x_w, d_x[:, c0:c0 + WIN])
                nc.sync.dma_start(m_w, d_m[:, c0:c0 + WIN])
                nc.sync.dma_start(dt_w, d_dt[:, c0:c0 + WIN])

                delta = mid_pool.tile([P, WIN], F32, tag="delta")
                sdt = mid_pool.tile([P, WIN], F32, tag="sdt")
                nc.scalar.activation(delta, dt_w, AF.Sigmoid, scale=-1.0)
                nc.scalar.activation(sdt, dt_w, AF.Sigmoid)

                xhat = mid_pool.tile([P, WIN], F32, tag="xhat")
                if x_mean_zero:
                    nc.vector.tensor_mul(xhat, m_w, x_w)
                else:
                    t1 = mid_pool.tile([P, WIN], F32, tag="xc")
                    nc.vector.tensor_scalar(t1, x_w, s_xmean[:, :], None, mybir.AluOpType.subtract)
                    nc.vector.tensor_mul(t1, m_w, t1)
                    nc.vector.tensor_scalar(xhat, t1, s_xmean[:, :], None, mybir.AluOpType.add)

                # h_til = tanh(Wd @ xhat + bd); c = (1-delta)*h_til
                htp = ps_tmp.tile([H, WIN // 2], F32, tag="pst")
                htp2 = ps_tmp.tile([H, WIN // 2], F32, tag="pst")
                nc.tensor.matmul(htp, s_wdT, xhat[:, :WIN // 2], start=True, stop=True, skip_group_check=True)
                nc.tensor.matmul(htp2, s_wdT, xhat[:, WIN // 2:], start=True, stop=True, skip_group_check=True)
                htil = mid_pool.tile([H, WIN], F32, tag="htil")
                nc.scalar.activation(htil[:, :WIN // 2], htp, AF.Tanh, bias=s_bd[:, :])
                nc.scalar.activation(htil[:, WIN // 2:], htp2, AF.Tanh, bias=s_bd[:, :])
                c_w = mid_pool.tile([H, WIN], F32, tag="c")
                nc.vector.tensor_mul(c_w, sdt, htil)

                # a_n = gi_n + bih_n (SBUF resident, full window)
                anp = ps_tmp.tile([H, WIN // 2], F32, tag="pst")
                anp2 = ps_tmp.tile([H, WIN // 2], F32, tag="pst")
                nc.tensor.matmul(anp, s_wih["n", "x"], xhat[:, :WIN // 2], start=True, stop=False, skip_group_check=True)
                nc.tensor.matmul(anp, s_wih["n", "m"], m_w[:, :WIN // 2], start=False, stop=True, skip_group_check=True)
                nc.tensor.matmul(anp2, s_wih["n", "x"], xhat[:, WIN // 2:], start=True, stop=False, skip_group_check=True)
                nc.tensor.matmul(anp2, s_wih["n", "m"], m_w[:, WIN // 2:], start=False, stop=True, skip_group_check=True)
                a_n = mid_pool.tile([H, WIN], F32, tag="an")
                nc.vector.tensor_scalar(a_n[:, :WIN // 2], anp, s_bih_n[:, :], None, mybir.AluOpType.add)
                nc.vector.tensor_scalar(a_n[:, WIN // 2:], anp2, s_bih_n[:, :], None, mybir.AluOpType.add)
                return delta, c_w, a_n, xhat, m_w

            def psum_windows(tiles, j):
                """Gate-input matmuls for PSUM window j (WP steps) of a bulk window."""
                delta, c_w, a_n, xhat, m_w = tiles
                # [P, W, 2, HB] views of the bulk tiles
                xv = xhat.rearrange("p (t h b) -> p t h b", t=W, h=2)
                mv = m_w.rearrange("p (t h b) -> p t h b", t=W, h=2)
                cv = c_w.rearrange("p (t h b) -> p t h b", t=W, h=2)
                t0 = j * WP
                cbs = []
                for h, pool in ((0, ps_cbA), (1, ps_cbB)):
                    cb = pool.tile([H, 3, NSEC], F32, tag=f"cb{h}", name=f"cb{h}")
                    xs = xv[:, t0:t0 + WP, h, :]
                    ms = mv[:, t0:t0 + WP, h, :]
                    cs_ = cv[:, t0:t0 + WP, h, :]
                    nc.tensor.matmul(cb[:, :, :], s_bias3, s_ind3, start=True, stop=False, skip_group_check=True)
                    nc.tensor.matmul(cb[:, 0, :], s_wih["r", "x"], xs, start=False, stop=False, skip_group_check=True)
                    nc.tensor.matmul(cb[:, 0, :], s_wih["r", "m"], ms, start=False, stop=False, skip_group_check=True)
                    nc.tensor.matmul(cb[:, 0, :], s_whhT["r"], cs_, start=False, stop=False, skip_group_check=True)
                    nc.tensor.matmul(cb[:, 1, :], s_wih["z", "x"], xs, start=False, stop=False, skip_group_check=True)
                    nc.tensor.matmul(cb[:, 1, :], s_wih["z", "m"], ms, start=False, stop=False, skip_group_check=True)
                    nc.tensor.matmul(cb[:, 1, :], s_whhT["z"], cs_, start=False, stop=False, skip_group_check=True)
                    nc.tensor.matmul(cb[:, 2, :], s_whhT["n"], cs_, start=False, stop=False, skip_group_check=True)
                    cbs.append(cb)
                return cbs

            def chain(w, j, tiles, cbs):
                """WP serial steps for PSUM window (w, j); half B lags half A
                by one step so the two dependency chains stagger across
                engines instead of stalling in lockstep."""
                delta, c_w, a_n, xhat, m_w = tiles

                def half_step(h, k):
                    g = (w * W) + j * WP + k
                    col = (j * WP + k) * BLOC
                    cs = col + h * HB
                    cb = cbs[h]
                    if g == 0:
                        hpre = c_w[:, cs:cs + HB]
                    else:
                        hprev = H_sb[:, (g - 1) * BLOC + h * HB:(g - 1) * BLOC + h * HB + HB]
                        u = st_pool.tile([H, HB], F32, tag=f"u{h}", name=f"u{h}")
                        nc.vector.tensor_mul(u, delta[:, cs:cs + HB], hprev)
                        for gi_, wname in ((0, "r"), (1, "z"), (2, "n")):
                            nc.tensor.matmul(cb[:, gi_, k * HB:(k + 1) * HB], s_whhT[wname], u,
                                             start=False, stop=(k == WP - 1), skip_group_check=True)
                        hpre_t = st_pool.tile([H, HB], F32, tag=f"hp{h}", name=f"hp{h}")
                        nc.vector.tensor_add(hpre_t, u, c_w[:, cs:cs + HB])
                        hpre = hpre_t[:, :]
                    rz_sb = st_pool.tile([H, 2, HB], F32, tag=f"rz{h}", name=f"rz{h}")
                    nc.scalar.activation(rz_sb, cb[:, 0:2, k * HB:(k + 1) * HB], AF.Sigmoid)
                    tmp = st_pool.tile([H, HB], F32, tag=f"tm{h}", name=f"tm{h}")
                    nc.vector.tensor_mul(tmp, rz_sb[:, 0, :], cb[:, 2, k * HB:(k + 1) * HB])
                    wn = st_pool.tile([H, HB], F32, tag=f"wn{h}", name=f"wn{h}")
                    nc.vector.tensor_add(wn, tmp, a_n[:, cs:cs + HB])
                    q_sb = st_pool.tile([H, HB], F32, tag=f"q{h}", name=f"q{h}")
                    nc.vector.tensor_mul(q_sb, rz_sb[:, 1, :], hpre)
                    zp = st_pool.tile([H, HB], F32, tag=f"zp{h}", name=f"zp{h}")
                    nc.vector.tensor_scalar(zp, rz_sb[:, 1, :], -1.0, 1.0, mybir.AluOpType.mult, mybir.AluOpType.add)
                    n_sb = st_pool.tile([H, HB], F32, tag=f"ns{h}", name=f"ns{h}")
                    nc.scalar.activation(n_sb, wn, AF.Tanh)
                    t3 = st_pool.tile([H, HB], F32, tag=f"t3{h}", name=f"t3{h}")
                    nc.vector.tensor_mul(t3, zp, n_sb)
                    nc.vector.tensor_add(H_sb[:, g * BLOC + h * HB:g * BLOC + h * HB + HB], t3, q_sb)

                for k in range(WP):
                    half_step(0, k)
                    if k > 0:
                        half_step(1, k - 1)
                half_step(1, WP - 1)

            tiles = precompute(0)
            cbs = psum_windows(tiles, 0)
            for w in range(NW):
                for j in range(W // WP):
                    if j + 1 < W // WP:
                        nxt_cbs = psum_windows(tiles, j + 1)
                        nxt_tiles = tiles
                    elif w + 1 < NW:
                        nxt_tiles = precompute(w + 1)
                        nxt_cbs = psum_windows(nxt_tiles, 0)
                    else:
                        nxt_tiles = nxt_cbs = None
                    chain(w, j, tiles, cbs)
                    if j + 1 == W // WP:
                        nc.sync.dma_start(d_hout[:, w * WIN:(w + 1) * WIN], H_sb[:, w * WIN:(w + 1) * WIN])
                    tiles, cbs = nxt_tiles, nxt_cbs

        # ---- H_agg gather (gpsimd) ----
        NAGG = BLOC * TA  # 2048
        hagg_sb = singles.tile([H, NAGG], F32)
        s_gidx = singles.tile([P, NAGG // 16], mybir.dt.int16)
        nc.sync.dma_start(s_gidx, d_gidx[:])
        nc.gpsimd.load_library(library_config.ap_gather)
        nc.gpsimd.ap_gather(hagg_sb[:, :], H_sb[:, :], s_gidx[:, :],
                            channels=P, num_elems=NCOL + PAD, d=1, num_idxs=NAGG)
        nc.sync.dma_start(d_hagg[:], hagg_sb)

        # ---- head: eta = relu(feats @ W1.T + b1) @ W2.T ----
        s_w1hT = singles.tile([H, HEAD_H], F32)
        nc.sync.dma_start(s_w1hT, d_w1hT[:])
        s_w1sT = singles.tile([P_STD, HEAD_H], F32)
        nc.sync.dma_start(s_w1sT, d_w1sT[:])
        s_w1zT = singles.tile([P_STATIC, HEAD_H], F32)
        nc.sync.dma_start(s_w1zT, d_w1zT[:])
        s_b1 = singles.tile([1, HEAD_H], F32)
        nc.sync.dma_start(s_b1, d_b1[:])
        s_w2T = singles.tile([HEAD_H, 1], F32)
        nc.sync.dma_start(s_w2T, d_w2T[:])
        s_stdT = singles.tile([P_STD, NAGG], F32)
        nc.sync.dma_start(s_stdT, d_stdT[:])
        s_zT = singles.tile([P_STATIC, NAGG], F32)
        nc.sync.dma_start(s_zT, d_zT[:])
        s_ones_h = singles.tile([1, 512], F32)
        nc.vector.memset(s_ones_h, 1.0)

        r1_sb = singles.tile([HEAD_H, NAGG], F32)
        eta_sb = singles.tile([1, NAGG], F32)
        with ExitStack() as hctx:
            ps_head = hctx.enter_context(tc.tile_pool(name="ps_head", bufs=2, space="PSUM"))
            ps_eta = hctx.enter_context(tc.tile_pool(name="ps_eta", bufs=2, space="PSUM"))
            for j in range(NAGG // 512):
                sl = slice(j * 512, (j + 1) * 512)
                hp = ps_head.tile([HEAD_H, 512], F32, tag="hp")
                nc.tensor.matmul(hp, s_b1, s_ones_h, start=True, stop=False, skip_group_check=True)
                nc.tensor.matmul(hp, s_w1hT, hagg_sb[:, sl], start=False, stop=False, skip_group_check=True)
                nc.tensor.matmul(hp, s_w1sT, s_stdT[:, sl], start=False, stop=False, skip_group_check=True)
                nc.tensor.matmul(hp, s_w1zT, s_zT[:, sl], start=False, stop=True, skip_group_check=True)
                nc.scalar.activation(r1_sb[:, sl], hp, AF.Relu)
                ep = ps_eta.tile([1, 512], F32, tag="ep")
                nc.tensor.matmul(ep, s_w2T, r1_sb[:, sl], start=True, stop=True, skip_group_check=True)
                nc.vector.tensor_copy(eta_sb[:, sl], ep)
        nc.sync.dma_start(d_eta[:], eta_sb)

    nc.finalize()
    return nc


def _get_nc(x_mean_zero: bool):
    key = ("v1", x_mean_zero)
    if key not in _BUILD_CACHE:
        _BUILD_CACHE[key] = _build(x_mean_zero)
    return _BUILD_CACHE[key]


def _prep(X_raw, M_raw, DT_raw, STD_agg, Z, idx_map, x_mean,
          Wd, bd, Wih, bih, Whh, bhh, W1, b1, W2):
    X_raw = np.ascontiguousarray(np.asarray(X_raw, dtype=np.float32))
    M_raw = np.ascontiguousarray(np.asarray(M_raw, dtype=np.float32))
    DT_raw = np.ascontiguousarray(np.asarray(DT_raw, dtype=np.float32))
    STD_agg = np.asarray(STD_agg, dtype=np.float32)
    Z = np.asarray(Z, dtype=np.float32)
    idx_np = np.asarray(idx_map)
    x_mean = np.asarray(x_mean, dtype=np.float32)
    Wd = np.asarray(Wd, dtype=np.float32)
    bd = np.asarray(bd, dtype=np.float32)
    Wih = np.asarray(Wih, dtype=np.float32)
    bih = np.asarray(bih, dtype=np.float32)
    Whh = np.asarray(Whh, dtype=np.float32)
    bhh = np.asarray(bhh, dtype=np.float32)
    W1 = np.asarray(W1, dtype=np.float32)
    b1 = np.asarray(b1, dtype=np.float32)
    W2 = np.asarray(W2, dtype=np.float32)

    x_mean_zero = not np.any(x_mean)

    bih_r, bih_z, bih_n = bih[:H], bih[H:2 * H], bih[2 * H:]
    bhh_r, bhh_z, bhh_n = bhh[:H], bhh[H:2 * H], bhh[2 * H:]
    shared = {
        "wdT": np.ascontiguousarray(Wd.T),
        "wih_rx": np.ascontiguousarray(Wih[0:H, :P].T),
        "wih_rm": np.ascontiguousarray(Wih[0:H, P:].T),
        "wih_zx": np.ascontiguousarray(Wih[H:2 * H, :P].T),
        "wih_zm": np.ascontiguousarray(Wih[H:2 * H, P:].T),
        "wih_nx": np.ascontiguousarray(Wih[2 * H:, :P].T),
        "wih_nm": np.ascontiguousarray(Wih[2 * H:, P:].T),
        "whhT_r": np.ascontiguousarray(Whh[0:H].T),
        "whhT_z": np.ascontiguousarray(Whh[H:2 * H].T),
        "whhT_n": np.ascontiguousarray(Whh[2 * H:].T),
        "bias3": np.ascontiguousarray(np.stack([bih_r + bhh_r, bih_z + bhh_z, bhh_n])),
        "ind3": np.ascontiguousarray((np.arange(3 * 128)[None, :] // 128 == np.arange(3)[:, None]).astype(np.float32)),
        "bih_n": np.ascontiguousarray(bih_n[:, None]),
        "bd": np.ascontiguousarray(bd[:, None]),
        "xmean": np.ascontiguousarray(x_mean[:, None]),
        "w1hT": np.ascontiguousarray(W1[:, :H].T),
        "w1sT": np.ascontiguousarray(W1[:, H:H + P_STD].T),
        "w1zT": np.ascontiguousarray(W1[:, H + P_STD:].T),
        "b1": np.ascontiguousarray(b1[None, :]),
        "w2T": np.ascontiguousarray(W2.T),
    }

    in_maps = []
    for c in range(NCORES):
        bs = slice(c * BLOC, (c + 1) * BLOC)
        xc = np.ascontiguousarray(X_raw[bs].transpose(2, 1, 0).reshape(P, NCOL))
        mc = np.ascontiguousarray(M_raw[bs].transpose(2, 1, 0).reshape(P, NCOL))
        dtc = np.ascontiguousarray(DT_raw[bs].transpose(2, 1, 0).reshape(P, NCOL))
        idxc = idx_np[bs].astype(np.int64)
        valid = idxc >= 0
        safe = np.clip(idxc, 0, T - 1)
        cols = safe * BLOC + np.arange(BLOC)[:, None]
        cols = np.where(valid, cols, NCOL).astype(np.int64).reshape(-1)
        gidx16 = np.zeros((16, len(cols) // 16), dtype=np.int16)
        for j, v in enumerate(cols):
            gidx16[j % 16, j // 16] = v
        gidx = np.ascontiguousarray(np.tile(gidx16, (8, 1)))
        stdT = np.ascontiguousarray(STD_agg[bs].transpose(2, 0, 1).reshape(P_STD, BLOC * TA))
        zT = np.ascontiguousarray(
            np.repeat(Z[bs].T[:, :, None], TA, axis=2).reshape(P_STATIC, BLOC * TA))
        im = {"x": xc, "m": mc, "dt": dtc, "gidx": gidx, "stdT": stdT, "zT": zT}
        im.update(shared)
        in_maps.append(im)
    return in_maps, idx_np, x_mean_zero


def _post(outs, idx_np):
    H_raw = np.empty((B, T, H), dtype=np.float32)
    H_agg = np.empty((B, TA, H), dtype=np.float32)
    eta = np.empty((B, TA), dtype=np.float32)
    for c in range(NCORES):
        bs = slice(c * BLOC, (c + 1) * BLOC)
        ho = outs[c]["h_out"].reshape(H, T, BLOC)
        H_raw[bs] = ho.transpose(2, 1, 0)
        ha = outs[c]["hagg_out"].reshape(H, BLOC, TA)
        H_agg[bs] = ha.transpose(1, 2, 0)
        eta[bs] = outs[c]["eta_out"].reshape(BLOC, TA)
    mask_sel = (idx_np >= 0).astype(np.float32)
    return eta, H_raw, H_agg, mask_sel


def kernel(**inputs):
    in_maps, idx_np, x_mean_zero = _prep(**inputs)
    nc = _get_nc(x_mean_zero)
    res = run_bass_kernel_spmd(nc, in_maps, list(range(NCORES)))
    return _post(res.results, idx_np)


# revision 20
# speedup vs baseline: 59.6116x; 1.0041x over previous
"""Trainium2 Bass kernel for nn_DeepPSDual (masked-decay GRU + gather head).

Contract: kernel(**inputs) takes FULL unsharded inputs (as produced by the
problem's setup_inputs) and returns the full outputs
(eta, H_raw, H_agg, mask_sel) exactly like the reference.

Sharding: pure data parallel over batch B=256 -> 8 cores x 32 batches.
All weights replicated; the T=512 recurrence stays local per core.

Device layout trick: everything on device lives in [feature-on-partitions,
(t*32+b) on free] layout; the host does all transposes (cheap numpy) so the
device never transposes anything.

Key algebra used (exact):
  delta = exp(-softplus(dt)) == sigmoid(-dt)
  1 - delta == sigmoid(dt)
  M_raw is a 0/1 mask  =>  x_dec == x_hat  (both equal m*x + (1-m)*x_mean)
  h_pre_t = delta_t*h_{t-1} + c_t,   c_t = (1-delta_t)*h_til_t
  gh_t = h_pre_t @ Whh.T + bhh  -> fed as h_pre @ WhhT accumulated onto
  PSUM-resident precomputed gate inputs (gi parts + biases).
"""

import os
import sys
from contextlib import ExitStack

import numpy as np

for _p in ("/opt/trn_rl_repo", "/root/.axon_site/_ro/trn_rl_repo"):
    if os.path.isdir(_p) and _p not in sys.path:
        sys.path.insert(0, _p)

import concourse.bass as bass  # noqa: E402
import concourse.bacc as bacc  # noqa: E402
import concourse.tile as tile  # noqa: E402
from concourse import mybir  # noqa: E402
from concourse.bass_utils import run_bass_kernel_spmd  # noqa: E402
from concourse import library_config  # noqa: E402

F32 = mybir.dt.float32
AF = mybir.ActivationFunctionType

NCORES = 8
B = 256
BLOC = B // NCORES  # 32
T = 512
P = 128  # feature dim == hidden dim
H = 128
TA = 64  # aggregated slots
P_STD = 32
P_STATIC = 16
HEAD_H = 64
W = 16  # bulk precompute window (steps)
WP = 4  # PSUM window (steps per bank set)
NCOL = T * BLOC  # 16384 columns per core
PAD = 32  # zero columns appended to H for masked gather
NW = T // W

_BUILD_CACHE = {}


def _build(x_mean_zero: bool):
    nc = bacc.Bacc("TRN2", debug=False)

    # ---- DRAM I/O (per core) ----
    d_x = nc.dram_tensor("x", [P, NCOL], F32, kind="ExternalInput")
    d_m = nc.dram_tensor("m", [P, NCOL], F32, kind="ExternalInput")
    d_dt = nc.dram_tensor("dt", [P, NCOL], F32, kind="ExternalInput")
    d_wdT = nc.dram_tensor("wdT", [P, H], F32, kind="ExternalInput")
    d_wih = {}
    for g in ("r", "z", "n"):
        for c in ("x", "m"):
            d_wih[g, c] = nc.dram_tensor(f"wih_{g}{c}", [P, H], F32, kind="ExternalInput")
    d_whhT = {g: nc.dram_tensor(f"whhT_{g}", [H, H], F32, kind="ExternalInput") for g in ("r", "z", "n")}
    d_bias3 = nc.dram_tensor("bias3", [3, H], F32, kind="ExternalInput")  # [bih_r+bhh_r; bih_z+bhh_z; bhh_n]
    d_ind3 = nc.dram_tensor("ind3", [3, 3 * WP * 16], F32, kind="ExternalInput")  # section indicators
    d_bih_n = nc.dram_tensor("bih_n", [H, 1], F32, kind="ExternalInput")
    d_bd = nc.dram_tensor("bd", [H, 1], F32, kind="ExternalInput")
    d_xmean = nc.dram_tensor("xmean", [P, 1], F32, kind="ExternalInput")
    d_gidx = nc.dram_tensor("gidx", [P, (BLOC * TA) // 16], mybir.dt.int16, kind="ExternalInput")
    d_w1hT = nc.dram_tensor("w1hT", [H, HEAD_H], F32, kind="ExternalInput")
    d_w1sT = nc.dram_tensor("w1sT", [P_STD, HEAD_H], F32, kind="ExternalInput")
    d_w1zT = nc.dram_tensor("w1zT", [P_STATIC, HEAD_H], F32, kind="ExternalInput")
    d_b1 = nc.dram_tensor("b1", [1, HEAD_H], F32, kind="ExternalInput")
    d_w2T = nc.dram_tensor("w2T", [HEAD_H, 1], F32, kind="ExternalInput")
    d_stdT = nc.dram_tensor("stdT", [P_STD, BLOC * TA], F32, kind="ExternalInput")
    d_zT = nc.dram_tensor("zT", [P_STATIC, BLOC * TA], F32, kind="ExternalInput")

    d_hout = nc.dram_tensor("h_out", [H, NCOL], F32, kind="ExternalOutput")
    d_hagg = nc.dram_tensor("hagg_out", [H, BLOC * TA], F32, kind="ExternalOutput")
    d_eta = nc.dram_tensor("eta_out", [1, BLOC * TA], F32, kind="ExternalOutput")

    with tile.TileContext(nc) as tc, ExitStack() as ctx:
        singles = ctx.enter_context(tc.tile_pool(name="singles", bufs=1))

        # persistent SBUF tensors
        H_sb = singles.tile([H, NCOL + PAD], F32)
        nc.vector.memset(H_sb[:, NCOL:], 0.0)

        s_wdT = singles.tile([P, H], F32)
        nc.sync.dma_start(s_wdT, d_wdT[:])
        s_wih = {}
        for k, d in d_wih.items():
            s_wih[k] = singles.tile([P, H], F32, name=f"wih_{k[0]}{k[1]}", tag=f"wih_{k[0]}{k[1]}")
            nc.sync.dma_start(s_wih[k], d[:])
        s_whhT = {}
        for g, d in d_whhT.items():
            s_whhT[g] = singles.tile([H, H], F32, name=f"whhT_{g}", tag=f"whhT_{g}")
            nc.sync.dma_start(s_whhT[g], d[:])
        s_bias3 = singles.tile([3, H], F32)
        nc.sync.dma_start(s_bias3, d_bias3[:])
        s_ind3 = singles.tile([3, 3 * WP * 16], F32)
        nc.sync.dma_start(s_ind3, d_ind3[:])
        s_bih_n = singles.tile([H, 1], F32)
        nc.sync.dma_start(s_bih_n, d_bih_n[:])
        s_bd = singles.tile([H, 1], F32)
        nc.sync.dma_start(s_bd, d_bd[:])
        s_xmean = singles.tile([P, 1], F32)
        nc.sync.dma_start(s_xmean, d_xmean[:])

        WIN = W * BLOC  # 512 columns per bulk window
        HB = BLOC // 2  # 16 batches per half-chain
        NSEC = WP * HB  # 128 columns per gate section in a PSUM bank

        with ExitStack() as chain_ctx:
            inp_pool = chain_ctx.enter_context(tc.tile_pool(name="inp", bufs=2))
            mid_pool = chain_ctx.enter_context(tc.tile_pool(name="mid", bufs=2))
            ps_cbA = chain_ctx.enter_context(tc.tile_pool(name="ps_cbA", bufs=3, space="PSUM"))
            ps_cbB = chain_ctx.enter_context(tc.tile_pool(name="ps_cbB", bufs=3, space="PSUM"))
            ps_tmp = chain_ctx.enter_context(tc.tile_pool(name="ps_tmp", bufs=2, space="PSUM"))
            st_pool = chain_ctx.enter_context(tc.tile_pool(name="step", bufs=4))

            def precompute(w):
                """DMA + bulk elementwise for bulk window w (W steps)."""
                c0 = w * WIN
                x_w = inp_pool.tile([P, WIN], F32, tag="x")
                m_w = inp_pool.tile([P, WIN], F32, tag="m")
                dt_w = inp_pool.tile([P, WIN], F32, tag="dt")
                nc.sync.dma_start(x_w, d_x[:, c0:c0 + WIN])
                nc.sync.dma_start(m_w, d_m[:, c0:c0 + WIN])
                nc.sync.dma_start(dt_w, d_dt[:, c0:c0 + WIN])

                delta = mid_pool.tile([P, WIN], F32, tag="delta")
                sdt = mid_pool.tile([P, WIN], F32, tag="sdt")
                nc.scalar.activation(delta, dt_w, AF.Sigmoid, scale=-1.0)
                nc.scalar.activation(sdt, dt_w, AF.Sigmoid)

                xhat = mid_pool.tile([P, WIN], F32, tag="xhat")
                if x_mean_zero:
                    nc.vector.tensor_mul(xhat, m_w, x_w)
                else:
                    t1 = mid_pool.tile([P, WIN], F32, tag="xc")
                    nc.vector.tensor_scalar(t1, x_w, s_xmean[:, :], None, mybir.AluOpType.subtract)
                    nc.vector.tensor_mul(t1, m_w, t1)
                    nc.vector.tensor_scalar(xhat, t1, s_xmean[:, :], None, mybir.AluOpType.add)

                # h_til = tanh(Wd @ xhat + bd); c = (1-delta)*h_til
                htp = ps_tmp.tile([H, WIN // 2], F32, tag="pst")
                htp2 = ps_tmp.tile([H, WIN // 2], F32, tag="pst")
                nc.tensor.matmul(htp, s_wdT, xhat[:, :WIN // 2], start=True, stop=True, skip_group_check=True)
                nc.tensor.matmul(htp2, s_wdT, xhat[:, WIN // 2:], start=True, stop=True, skip_group_check=True)
                htil = mid_pool.tile([H, WIN], F32, tag="htil")
                nc.scalar.activation(htil[:, :WIN // 2], htp, AF.Tanh, bias=s_bd[:, :])
                nc.scalar.activation(htil[:, WIN // 2:], htp2, AF.Tanh, bias=s_bd[:, :])
                c_w = mid_pool.tile([H, WIN], F32, tag="c")
                nc.vector.tensor_mul(c_w, sdt, htil)

                # a_n = gi_n + bih_n (SBUF resident, full window)
                anp = ps_tmp.tile([H, WIN // 2], F32, tag="pst")
                anp2 = ps_tmp.tile([H, WIN // 2], F32, tag="pst")
                nc.tensor.matmul(anp, s_wih["n", "x"], xhat[:, :WIN // 2], start=True, stop=False, skip_group_check=True)
                nc.tensor.matmul(anp, s_wih["n", "m"], m_w[:, :WIN // 2], start=False, stop=True, skip_group_check=True)
                nc.tensor.matmul(anp2, s_wih["n", "x"], xhat[:, WIN // 2:], start=True, stop=False, skip_group_check=True)
                nc.tensor.matmul(anp2, s_wih["n", "m"], m_w[:, WIN // 2:], start=False, stop=True, skip_group_check=True)
                a_n = mid_pool.tile([H, WIN], F32, tag="an")
                nc.vector.tensor_scalar(a_n[:, :WIN // 2], anp, s_bih_n[:, :], None, mybir.AluOpType.add)
                nc.vector.tensor_scalar(a_n[:, WIN // 2:], anp2, s_bih_n[:, :], None, mybir.AluOpType.add)
                return delta, c_w, a_n, xhat, m_w

            def psum_windows(tiles, j):
                """Gate-input matmuls for PSUM window j (WP steps) of a bulk window."""
                delta, c_w, a_n, xhat, m_w = tiles
                # [P, W, 2, HB] views of the bulk tiles
                xv = xhat.rearrange("p (t h b) -> p t h b", t=W, h=2)
                mv = m_w.rearrange("p (t h b) -> p t h b", t=W, h=2)
                cv = c_w.rearrange("p (t h b) -> p t h b", t=W, h=2)
                t0 = j * WP
                cbs = []
                for h, pool in ((0, ps_cbA), (1, ps_cbB)):
                    cb = pool.tile([H, 3, NSEC], F32, tag=f"cb{h}", name=f"cb{h}")
                    xs = xv[:, t0:t0 + WP, h, :]
                    ms = mv[:, t0:t0 + WP, h, :]
                    cs_ = cv[:, t0:t0 + WP, h, :]
                    nc.tensor.matmul(cb[:, :, :], s_bias3, s_ind3, start=True, stop=False, skip_group_check=True)
                    nc.tensor.matmul(cb[:, 0, :], s_wih["r", "x"], xs, start=False, stop=False, skip_group_check=True)
                    nc.tensor.matmul(cb[:, 0, :], s_wih["r", "m"], ms, start=False, stop=False, skip_group_check=True)
                    nc.tensor.matmul(cb[:, 0, :], s_whhT["r"], cs_, start=False, stop=False, skip_group_check=True)
                    nc.tensor.matmul(cb[:, 1, :], s_wih["z", "x"], xs, start=False, stop=False, skip_group_check=True)
                    nc.tensor.matmul(cb[:, 1, :], s_wih["z", "m"], ms, start=False, stop=False, skip_group_check=True)
                    nc.tensor.matmul(cb[:, 1, :], s_whhT["z"], cs_, start=False, stop=False, skip_group_check=True)
                    nc.tensor.matmul(cb[:, 2, :], s_whhT["n"], cs_, start=False, stop=False, skip_group_check=True)
                    cbs.append(cb)
                return cbs

            def chain(w, j, tiles, cbs):
                """WP serial steps for PSUM window (w, j); half B lags half A
                by one step so the two dependency chains stagger across
                engines instead of stalling in lockstep."""
                delta, c_w, a_n, xhat, m_w = tiles

                def half_step(h, k):
                    g = (w * W) + j * WP + k
                    col = (j * WP + k) * BLOC
                    cs = col + h * HB
                    cb = cbs[h]
                    if g == 0:
                        hpre = c_w[:, cs:cs + HB]
                    else:
                        hprev = H_sb[:, (g - 1) * BLOC + h * HB:(g - 1) * BLOC + h * HB + HB]
                        u = st_pool.tile([H, HB], F32, tag=f"u{h}", name=f"u{h}")
                        nc.vector.tensor_mul(u, delta[:, cs:cs + HB], hprev)
                        for gi_, wname in ((0, "r"), (1, "z"), (2, "n")):
                            nc.tensor.matmul(cb[:, gi_, k * HB:(k + 1) * HB], s_whhT[wname], u,
                                             start=False, stop=(k == WP - 1), skip_group_check=True)
                        hpre_t = st_pool.tile([H, HB], F32, tag=f"hp{h}", name=f"hp{h}")
                        nc.vector.tensor_add(hpre_t, u, c_w[:, cs:cs + HB])
                        hpre = hpre_t[:, :]
                    rz_sb = st_pool.tile([H, 2, HB], F32, tag=f"rz{h}", name=f"rz{h}")
                    nc.scalar.activation(rz_sb, cb[:, 0:2, k * HB:(k + 1) * HB], AF.Sigmoid)
                    tmp = st_pool.tile([H, HB], F32, tag=f"tm{h}", name=f"tm{h}")
                    nc.vector.tensor_mul(tmp, rz_sb[:, 0, :], cb[:, 2, k * HB:(k + 1) * HB])
                    wn = st_pool.tile([H, HB], F32, tag=f"wn{h}", name=f"wn{h}")
                    nc.vector.tensor_add(wn, tmp, a_n[:, cs:cs + HB])
                    q_sb = st_pool.tile([H, HB], F32, tag=f"q{h}", name=f"q{h}")
                    nc.vector.tensor_mul(q_sb, rz_sb[:, 1, :], hpre)
                    zp = st_pool.tile([H, HB], F32, tag=f"zp{h}", name=f"zp{h}")
                    nc.vector.tensor_scalar(zp, rz_sb[:, 1, :], -1.0, 1.0, mybir.AluOpType.mult, mybir.AluOpType.add)
                    n_sb = st_pool.tile([H, HB], F32, tag=f"ns{h}", name=f"ns{h}")
                    nc.scalar.activation(n_sb, wn, AF.Tanh)
                    t3 = st_pool.tile([H, HB], F32, tag=f"t3{h}", name=f"t3{h}")
                    nc.vector.tensor_mul(t3, zp, n_sb)
                    nc.vector.tensor_add(H_sb[:, g * BLOC + h * HB:g * BLOC + h * HB + HB], t3, q_sb)

                for k in range(WP):
                    half_step(0, k)
                    if k >= 2:
                        half_step(1, k - 2)
                half_step(1, WP - 2)
                half_step(1, WP - 1)

            tiles = precompute(0)
            cbs = psum_windows(tiles, 0)
            for w in range(NW):
                for j in range(W // WP):
                    if j + 1 < W // WP:
                        nxt_cbs = psum_windows(tiles, j + 1)
                        nxt_tiles = tiles
                    elif w + 1 < NW:
                        nxt_tiles = precompute(w + 1)
                        nxt_cbs = psum_windows(nxt_tiles, 0)
                    else:
                        nxt_tiles = nxt_cbs = None
                    chain(w, j, tiles, cbs)
                    if j + 1 == W // WP:
                        nc.sync.dma_start(d_hout[:, w * WIN:(w + 1) * WIN], H_sb[:, w * WIN:(w + 1) * WIN])
                    tiles, cbs = nxt_tiles, nxt_cbs

        # ---- H_agg gather (gpsimd) ----
        NAGG = BLOC * TA  # 2048
        hagg_sb = singles.tile([H, NAGG], F32)
        s_gidx = singles.tile([P, NAGG // 16], mybir.dt.int16)
        nc.sync.dma_start(s_gidx, d_gidx[:])
        nc.gpsimd.load_library(library_config.ap_gather)
        nc.gpsimd.ap_gather(hagg_sb[:, :], H_sb[:, :], s_gidx[:, :],
                            channels=P, num_elems=NCOL + PAD, d=1, num_idxs=NAGG)
        nc.sync.dma_start(d_hagg[:], hagg_sb)

        # ---- head: eta = relu(feats @ W1.T + b1) @ W2.T ----
        s_w1hT = singles.tile([H, HEAD_H], F32)
        nc.sync.dma_start(s_w1hT, d_w1hT[:])
        s_w1sT = singles.tile([P_STD, HEAD_H], F32)
        nc.sync.dma_start(s_w1sT, d_w1sT[:])
        s_w1zT = singles.tile([P_STATIC, HEAD_H], F32)
        nc.sync.dma_start(s_w1zT, d_w1zT[:])
        s_b1 = singles.tile([1, HEAD_H], F32)
        nc.sync.dma_start(s_b1, d_b1[:])
        s_w2T = singles.tile([HEAD_H, 1], F32)
        nc.sync.dma_start(s_w2T, d_w2T[:])
        s_stdT = singles.tile([P_STD, NAGG], F32)
        nc.sync.dma_start(s_stdT, d_stdT[:])
        s_zT = singles.tile([P_STATIC, NAGG], F32)
        nc.sync.dma_start(s_zT, d_zT[:])
        s_ones_h = singles.tile([1, 512], F32)
        nc.vector.memset(s_ones_h, 1.0)

        r1_sb = singles.tile([HEAD_H, NAGG], F32)
        eta_sb = singles.tile([1, NAGG], F32)
        with ExitStack() as hctx:
            ps_head = hctx.enter_context(tc.tile_pool(name="ps_head", bufs=2, space="PSUM"))
            ps_eta = hctx.enter_context(tc.tile_pool(name="ps_eta", bufs=2, space="PSUM"))
            for j in range(NAGG // 512):
                sl = slice(j * 512, (j + 1) * 512)
                hp = ps_head.tile([HEAD_H, 512], F32, tag="hp")
                nc.tensor.matmul(hp, s_b1, s_ones_h, start=True, stop=False, skip_group_check=True)
                nc.tensor.matmul(hp, s_w1hT, hagg_sb[:, sl], start=False, stop=False, skip_group_check=True)
                nc.tensor.matmul(hp, s_w1sT, s_stdT[:, sl], start=False, stop=False, skip_group_check=True)
                nc.tensor.matmul(hp, s_w1zT, s_zT[:, sl], start=False, stop=True, skip_group_check=True)
                nc.scalar.activation(r1_sb[:, sl], hp, AF.Relu)
                ep = ps_eta.tile([1, 512], F32, tag="ep")
                nc.tensor.matmul(ep, s_w2T, r1_sb[:, sl], start=True, stop=True, skip_group_check=True)
                nc.vector.tensor_copy(eta_sb[:, sl], ep)
        nc.sync.dma_start(d_eta[:], eta_sb)

    nc.finalize()
    return nc


def _get_nc(x_mean_zero: bool):
    key = ("v1", x_mean_zero)
    if key not in _BUILD_CACHE:
        _BUILD_CACHE[key] = _build(x_mean_zero)
    return _BUILD_CACHE[key]


def _prep(X_raw, M_raw, DT_raw, STD_agg, Z, idx_map, x_mean,
          Wd, bd, Wih, bih, Whh, bhh, W1, b1, W2):
    X_raw = np.ascontiguousarray(np.asarray(X_raw, dtype=np.float32))
    M_raw = np.ascontiguousarray(np.asarray(M_raw, dtype=np.float32))
    DT_raw = np.ascontiguousarray(np.asarray(DT_raw, dtype=np.float32))
    STD_agg = np.asarray(STD_agg, dtype=np.float32)
    Z = np.asarray(Z, dtype=np.float32)
    idx_np = np.asarray(idx_map)
    x_mean = np.asarray(x_mean, dtype=np.float32)
    Wd = np.asarray(Wd, dtype=np.float32)
    bd = np.asarray(bd, dtype=np.float32)
    Wih = np.asarray(Wih, dtype=np.float32)
    bih = np.asarray(bih, dtype=np.float32)
    Whh = np.asarray(Whh, dtype=np.float32)
    bhh = np.asarray(bhh, dtype=np.float32)
    W1 = np.asarray(W1, dtype=np.float32)
    b1 = np.asarray(b1, dtype=np.float32)
    W2 = np.asarray(W2, dtype=np.float32)

    x_mean_zero = not np.any(x_mean)

    bih_r, bih_z, bih_n = bih[:H], bih[H:2 * H], bih[2 * H:]
    bhh_r, bhh_z, bhh_n = bhh[:H], bhh[H:2 * H], bhh[2 * H:]
    shared = {
        "wdT": np.ascontiguousarray(Wd.T),
        "wih_rx": np.ascontiguousarray(Wih[0:H, :P].T),
        "wih_rm": np.ascontiguousarray(Wih[0:H, P:].T),
        "wih_zx": np.ascontiguousarray(Wih[H:2 * H, :P].T),
        "wih_zm": np.ascontiguousarray(Wih[H:2 * H, P:].T),
        "wih_nx": np.ascontiguousarray(Wih[2 * H:, :P].T),
        "wih_nm": np.ascontiguousarray(Wih[2 * H:, P:].T),
        "whhT_r": np.ascontiguousarray(Whh[0:H].T),
        "whhT_z": np.ascontiguousarray(Whh[H:2 * H].T),
        "whhT_n": np.ascontiguousarray(Whh[2 * H:].T),
        "bias3": np.ascontiguousarray(np.stack([bih_r + bhh_r, bih_z + bhh_z, bhh_n])),
        "ind3": np.ascontiguousarray((np.arange(3 * 128)[None, :] // 128 == np.arange(3)[:, None]).astype(np.float32)),
        "bih_n": np.ascontiguousarray(bih_n[:, None]),
        "bd": np.ascontiguousarray(bd[:, None]),
        "xmean": np.ascontiguousarray(x_mean[:, None]),
        "w1hT": np.ascontiguousarray(W1[:, :H].T),
        "w1sT": np.ascontiguousarray(W1[:, H:H + P_STD].T),
        "w1zT": np.ascontiguousarray(W1[:, H + P_STD:].T),
        "b1": np.ascontiguousarray(b1[None, :]),
        "w2T": np.ascontiguousarray(W2.T),
    }

    in_maps = []
    for c in range(NCORES):
        bs = slice(c * BLOC, (c + 1) * BLOC)
        xc = np.ascontiguousarray(X_raw[bs].transpose(2, 1, 0).reshape(P, NCOL))
        mc = np.ascontiguousarray(M_raw[bs].transpose(2, 1, 0).reshape(P, NCOL))
        dtc = np.ascontiguousarray(DT_raw[bs].transpose(2, 1, 0).reshape(P, NCOL))
        idxc = idx_np[bs].astype(np.int64)
        valid = idxc >= 0
        safe = np.clip(idxc, 0, T - 1)
        cols = safe * BLOC + np.arange(BLOC)[:, None]
        cols = np.where(valid, cols, NCOL).astype(np.int64).reshape(-1)
        gidx16 = np.zeros((16, len(cols) // 16), dtype=np.int16)
        for j, v in enumerate(cols):
            gidx16[j % 16, j // 16] = v
        gidx = np.ascontiguousarray(np.tile(gidx16, (8, 1)))
        stdT = np.ascontiguousarray(STD_agg[bs].transpose(2, 0, 1).reshape(P_STD, BLOC * TA))
        zT = np.ascontiguousarray(
            np.repeat(Z[bs].T[:, :, None], TA, axis=2).reshape(P_STATIC, BLOC * TA))
        im = {"x": xc, "m": mc, "dt": dtc, "gidx": gidx, "stdT": stdT, "zT": zT}
        im.update(shared)
        in_maps.append(im)
    return in_maps, idx_np, x_mean_zero


def _post(outs, idx_np):
    H_raw = np.empty((B, T, H), dtype=np.float32)
    H_agg = np.empty((B, TA, H), dtype=np.float32)
    eta = np.empty((B, TA), dtype=np.float32)
    for c in range(NCORES):
        bs = slice(c * BLOC, (c + 1) * BLOC)
        ho = outs[c]["h_out"].reshape(H, T, BLOC)
        H_raw[bs] = ho.transpose(2, 1, 0)
        ha = outs[c]["hagg_out"].reshape(H, BLOC, TA)
        H_agg[bs] = ha.transpose(1, 2, 0)
        eta[bs] = outs[c]["eta_out"].reshape(BLOC, TA)
    mask_sel = (idx_np >= 0).astype(np.float32)
    return eta, H_raw, H_agg, mask_sel


def kernel(**inputs):
    in_maps, idx_np, x_mean_zero = _prep(**inputs)
    nc = _get_nc(x_mean_zero)
    res = run_bass_kernel_spmd(nc, in_maps, list(range(NCORES)))
    return _post(res.results, idx_np)
